# revision 2
# baseline (speedup 1.0000x reference)
"""Row-sharded attention slab kernel, host-prepped fp8 Q/K/V.

Each of the 8 cores owns a [N/8, N] slab of the attention matrix.  The
host precomputes the three D x D projections in fp32 (O(N*D^2), ~5% of
total FLOPs), pre-normalizes k rows, and ships fp8 tensors: qT8 (own
rows), kT8 (all columns, pre-normalized), and v8 (row-major V with a
trailing ones column that accumulates the softmax-style denominator).

The device does the O(N^2*D) work: score matmuls (fp8 DoubleRow),
ReLU + fp8 cast of the [N/8, N] score slab (split across ACT and DVE,
the true bottleneck), and the w@v accumulation into PSUM, evacuated
once per 256-row block and DMA'd out as fp32 [num | den] rows.

The host then removes the diagonal term (emulating the device's fp8
rounding so the subtraction matches what the device accumulated),
divides by the row sums, and adds the residual x and the V bias (the
bias commutes out of the attention average because rows of w sum to 1).
"""

import numpy as np

import concourse.bass as bass
import concourse.bacc as bacc
import concourse.mybir as mybir
from concourse import tile
from concourse.bass_utils import run_bass_kernel_spmd

F32 = mybir.dt.float32
FP8 = mybir.dt.float8e4
AF = mybir.ActivationFunctionType
DR = mybir.MatmulPerfMode.DoubleRow

M = 8
N = 8192
D = 256

TRACE = False
LAST = None
_CACHE = {}

# ~47% of the 64 relu tiles go to DVE (1192ns/op) and the rest to ACT
# (1067ns/op) so both engines finish together.
N_DVE_RELU = 30


def _dve_set(total, n_dve):
    return {i for i in range(total) if (i * n_dve) // total != ((i + 1) * n_dve) // total}


def build(r=N // M):
    NP_ = N // 256           # 32 column pairs (v8 major dim)
    NCH = N // 1024          # 8 streaming chunks for kT8/v8
    NG = N // 512            # 16 score groups (4 col-blocks each) per row block
    RW = 256
    NRB = r // RW            # 4 row blocks

    nc = bacc.Bacc(None)
    qT8_d = nc.declare_dram_parameter("qT8", [128, 2, r], FP8, isOutput=False)
    kT8_d = nc.declare_dram_parameter("kT8", [128, 2, N], FP8, isOutput=False)
    v8_d = nc.declare_dram_parameter("v8", [128, NP_, 2, D + 1], FP8, isOutput=False)
    av_d = nc.declare_dram_parameter("av", [NRB, 128, 2, D + 1], F32, isOutput=True)

    dve_relu = _dve_set(NRB * NG, N_DVE_RELU)

    with tile.TileContext(nc, pool_alloc_mode="queue") as tc:
        with tc.tile_pool(name="pers", bufs=1) as pers, \
             tc.tile_pool(name="wtp", bufs=8) as wtp, \
             tc.tile_pool(name="avsp", bufs=2) as avsp, \
             tc.tile_pool(name="scp", bufs=2, space="PSUM") as scp, \
             tc.tile_pool(name="avp", bufs=2, space="PSUM") as avp:
            qT8 = pers.tile([128, 2, r], FP8, name="qT8", tag="qT8")
            kT8 = pers.tile([128, 2, N], FP8, name="kT8", tag="kT8")
            v8 = pers.tile([128, NP_, 2, D + 1], FP8, name="v8", tag="v8")

            # Stream inputs; first chunks ordered so compute can start early.
            nc.sync.dma_start(kT8[:, :, 0:1024], kT8_d[:, :, 0:1024])
            nc.sync.dma_start(qT8[:], qT8_d[:])
            nc.sync.dma_start(v8[:, 0:4, :, :], v8_d[:, 0:4, :, :])
            for ch in range(1, NCH):
                nc.sync.dma_start(kT8[:, :, ch * 1024:(ch + 1) * 1024],
                                  kT8_d[:, :, ch * 1024:(ch + 1) * 1024])
                nc.sync.dma_start(v8[:, ch * 4:(ch + 1) * 4, :, :],
                                  v8_d[:, ch * 4:(ch + 1) * 4, :, :])

            idx = 0
            for rb in range(NRB):
                # [128, 2, 512] so each 257-wide accumulator starts bank-aligned
                av = avp.tile([128, 2, 512], F32, name=f"av{rb}", tag="av")
                rsl = slice(rb * RW, (rb + 1) * RW)
                for g in range(NG):
                    sc = scp.tile([128, 1024], F32, name="sc", tag="sc")
                    for t in range(4):
                        jb = g * 4 + t
                        nc.tensor.matmul(sc[:, t * 256:(t + 1) * 256],
                                         kT8[:, :, jb * 128:(jb + 1) * 128],
                                         qT8[:, :, rsl],
                                         start=True, stop=True, perf_mode=DR)
                    wt = wtp.tile([128, 4, 256], FP8, name="wt", tag="wt")
                    if idx in dve_relu:
                        nc.vector.tensor_scalar_max(wt[:], sc[:], 0.0)
                    else:
                        nc.scalar.activation(wt[:], sc[:], AF.Relu)
                    idx += 1
                    for pair in range(2):
                        jp = g * 2 + pair
                        for s in range(2):
                            nc.tensor.matmul(av[:, s, 0:D + 1],
                                             wt[:, pair * 2:pair * 2 + 2,
                                                s * 128:(s + 1) * 128],
                                             v8[:, jp, :, :],
                                             start=(g == 0 and pair == 0),
                                             stop=(g == NG - 1 and pair == 1),
                                             perf_mode=DR)
                avs = avsp.tile([128, 2, D + 1], F32, name=f"avs{rb}", tag="avs")
                if rb % 2 == 0:
                    nc.vector.tensor_copy(avs[:], av[:, :, 0:D + 1])
                else:
                    nc.scalar.activation(avs[:], av[:, :, 0:D + 1], AF.Copy)
                nc.sync.dma_start(av_d[rb], avs[:])
    nc.compile()
    return nc


def _get_nc(r=N // M):
    if r not in _CACHE:
        _CACHE[r] = build(r)
    return _CACHE[r]


def _to_dr(a2d):
    c, f = a2d.shape
    return np.ascontiguousarray(a2d.reshape(2, 128, f).transpose(1, 0, 2))


def kernel(x, Wq, bq, Wk, bk, Wv, bv):
    global LAST
    np8 = mybir.dt.np(FP8)
    x = np.asarray(x, np.float32)
    n = x.shape[0]
    r = n // M

    q = x @ np.asarray(Wq, np.float32).T + np.asarray(bq, np.float32)
    k = x @ np.asarray(Wk, np.float32).T + np.asarray(bk, np.float32)
    v = x @ np.asarray(Wv, np.float32).T                      # bias added at the end
    kn = k / np.maximum(np.linalg.norm(k, axis=1, keepdims=True), 1e-12)

    q8 = q.astype(np8)
    k8 = kn.astype(np8)
    v8q = v.astype(np8)

    kT8 = _to_dr(np.ascontiguousarray(k8.T))
    vv = np.ones((n, D + 1), np8)
    vv[:, 0:D] = v8q
    v8t = np.ascontiguousarray(vv.reshape(n // 256, 2, 128, D + 1).transpose(2, 0, 1, 3))

    in_maps = []
    for c in range(M):
        rows = slice(c * r, (c + 1) * r)
        in_maps.append({
            "qT8": _to_dr(np.ascontiguousarray(q8[rows].T)),
            "kT8": kT8,
            "v8": v8t,
        })
    res = run_bass_kernel_spmd(_get_nc(r), in_maps, core_ids=list(range(M)), trace=TRACE)
    LAST = res

    av = np.concatenate(
        [res.results[c]["av"].transpose(0, 2, 1, 3).reshape(r, D + 1) for c in range(M)],
        axis=0).astype(np.float32)
    num = av[:, 0:D]
    den = av[:, D]

    # Remove the diagonal term, emulating the device's fp8 rounding of the
    # relu'd score and of v so the subtraction cancels what was accumulated.
    sdiag = np.maximum((q8.astype(np.float32) * k8.astype(np.float32)).sum(axis=1), 0.0)
    wdiag = sdiag.astype(np8).astype(np.float32)
    num = num - wdiag[:, None] * v8q.astype(np.float32)
    den = den - wdiag

    out = num / np.maximum(den, 1e-12)[:, None] + x + np.asarray(bv, np.float32)
    return np.ascontiguousarray(out, dtype=np.float32)


# revision 3
# speedup vs baseline: 1.3600x; 1.3600x over previous
"""Row-sharded attention slab kernel, host-prepped fp8 Q/K/V.

Each of the 8 cores owns a [N/8, N] slab of the attention matrix.  The
host precomputes the three D x D projections in fp32 (O(N*D^2), ~5% of
total FLOPs), pre-normalizes k rows, and ships fp8 tensors: qT8 (own
rows), kT8 (all columns, pre-normalized), and v8 (row-major V).

The device does the O(N^2*D) work: score matmuls (fp8 DoubleRow),
ReLU + fp8 cast of the [N/8, N] score slab (split across ACT and DVE,
the true bottleneck), and the w@v accumulation into PSUM.  The row-sum
denominator accumulates in a separate 1-bank PSUM tile via tiny
wt @ ones matmuls, which frees enough PSUM banks to triple-buffer the
score tiles - without that, the WAR edge (score matmuls for group g+2
overwriting the tile relu(g) reads) serializes the pipeline.  Emission
is software-pipelined so those prefetch matmuls precede the w@v
matmuls of the current group in PE order.

The host then removes the diagonal term (emulating the device's fp8
rounding so the subtraction matches what the device accumulated),
divides by the row sums, and adds the residual x and the V bias (the
bias commutes out of the attention average because rows of w sum to 1).
"""

import numpy as np

import concourse.bass as bass
import concourse.bacc as bacc
import concourse.mybir as mybir
from concourse import tile
from concourse.bass_utils import run_bass_kernel_spmd

F32 = mybir.dt.float32
FP8 = mybir.dt.float8e4
AF = mybir.ActivationFunctionType
DR = mybir.MatmulPerfMode.DoubleRow

M = 8
N = 8192
D = 256

TRACE = False
LAST = None
_CACHE = {}

# ~47% of the 64 relu tiles go to DVE (1192ns/op) and the rest to ACT
# (1067ns/op) so both engines finish together.
N_DVE_RELU = 30


def _dve_set(total, n_dve):
    return {i for i in range(total) if (i * n_dve) // total != ((i + 1) * n_dve) // total}


def build(r=N // M):
    NP_ = N // 256           # 32 column pairs (v8 major dim)
    NCH = N // 1024          # 8 streaming chunks for kT8/v8
    NG = N // 512            # 16 score groups (4 col-blocks each) per row block
    RW = 256
    NRB = r // RW            # 4 row blocks

    nc = bacc.Bacc(None)
    qT8_d = nc.declare_dram_parameter("qT8", [128, 2, r], FP8, isOutput=False)
    kT8_d = nc.declare_dram_parameter("kT8", [128, 2, N], FP8, isOutput=False)
    v8_d = nc.declare_dram_parameter("v8", [128, NP_, 2, D], FP8, isOutput=False)
    av_d = nc.declare_dram_parameter("av", [NRB, 128, 2, D + 1], F32, isOutput=True)

    seq = [(rb, g) for rb in range(NRB) for g in range(NG)]
    dve_relu = _dve_set(len(seq), N_DVE_RELU)

    with tile.TileContext(nc, pool_alloc_mode="queue") as tc:
        with tc.tile_pool(name="pers", bufs=1) as pers, \
             tc.tile_pool(name="wtp", bufs=8) as wtp, \
             tc.tile_pool(name="avsp", bufs=2) as avsp, \
             tc.tile_pool(name="scp", bufs=3, space="PSUM") as scp, \
             tc.tile_pool(name="avp", bufs=1, space="PSUM") as avp, \
             tc.tile_pool(name="denp", bufs=1, space="PSUM") as denp:
            qT8 = pers.tile([128, 2, r], FP8, name="qT8", tag="qT8")
            kT8 = pers.tile([128, 2, N], FP8, name="kT8", tag="kT8")
            v8 = pers.tile([128, NP_, 2, D], FP8, name="v8", tag="v8")
            ones8 = pers.tile([128, 2, 1], FP8, name="ones8", tag="ones8")
            den = denp.tile([128, 2 * NRB], F32, name="den", tag="den")

            nc.vector.memset(ones8[:], 1.0)

            # Stream inputs; first chunks ordered so compute can start early.
            nc.sync.dma_start(kT8[:, :, 0:1024], kT8_d[:, :, 0:1024])
            nc.sync.dma_start(qT8[:, :, 0:RW], qT8_d[:, :, 0:RW])
            nc.sync.dma_start(v8[:, 0:4, :, :], v8_d[:, 0:4, :, :])
            nc.sync.dma_start(qT8[:, :, RW:r], qT8_d[:, :, RW:r])
            for ch in range(1, NCH):
                nc.sync.dma_start(kT8[:, :, ch * 1024:(ch + 1) * 1024],
                                  kT8_d[:, :, ch * 1024:(ch + 1) * 1024])
                nc.sync.dma_start(v8[:, ch * 4:(ch + 1) * 4, :, :],
                                  v8_d[:, ch * 4:(ch + 1) * 4, :, :])

            sc_tiles = {}

            def emit_smm(i):
                rb, g = seq[i]
                sc = scp.tile([128, 1024], F32, name="sc", tag="sc")
                sc_tiles[i] = sc
                rsl = slice(rb * RW, (rb + 1) * RW)
                for t in range(4):
                    jb = g * 4 + t
                    nc.tensor.matmul(sc[:, t * 256:(t + 1) * 256],
                                     kT8[:, :, jb * 128:(jb + 1) * 128],
                                     qT8[:, :, rsl],
                                     start=True, stop=True, perf_mode=DR)

            avs = {}
            emit_smm(0)
            emit_smm(1)
            for i, (rb, g) in enumerate(seq):
                sc = sc_tiles.pop(i)
                wt = wtp.tile([128, 4, 256], FP8, name="wt", tag="wt")
                if i in dve_relu:
                    nc.vector.tensor_scalar_max(wt[:], sc[:], 0.0)
                else:
                    nc.scalar.activation(wt[:], sc[:], AF.Relu)
                if i + 2 < len(seq):
                    emit_smm(i + 2)
                if g == 0:
                    avs[rb] = avp.tile([128, 2, D], F32, name=f"av{rb}", tag="av")
                av = avs[rb]
                for pair in range(2):
                    jp = g * 2 + pair
                    st = (g == 0 and pair == 0)
                    sp = (g == NG - 1 and pair == 1)
                    for s in range(2):
                        wsl = wt[:, pair * 2:pair * 2 + 2, s * 128:(s + 1) * 128]
                        nc.tensor.matmul(av[:, s, :], wsl, v8[:, jp, :, :],
                                         start=st, stop=sp, perf_mode=DR)
                        nc.tensor.matmul(den[:, rb * 2 + s:rb * 2 + s + 1],
                                         wsl, ones8[:],
                                         start=st, stop=sp, perf_mode=DR)
                if g == NG - 1:
                    o = avsp.tile([128, 2, D + 1], F32, name=f"avs{rb}", tag="avs")
                    if rb % 2 == 0:
                        nc.vector.tensor_copy(o[:, :, 0:D], av[:])
                        nc.vector.tensor_copy(o[:, :, D], den[:, rb * 2:rb * 2 + 2])
                    else:
                        nc.scalar.activation(o[:, :, 0:D], av[:], AF.Copy)
                        nc.scalar.activation(o[:, :, D], den[:, rb * 2:rb * 2 + 2], AF.Copy)
                    nc.sync.dma_start(av_d[rb], o[:])
    nc.compile()
    return nc


def _get_nc(r=N // M):
    if r not in _CACHE:
        _CACHE[r] = build(r)
    return _CACHE[r]


def _to_dr(a2d):
    c, f = a2d.shape
    return np.ascontiguousarray(a2d.reshape(2, 128, f).transpose(1, 0, 2))


def kernel(x, Wq, bq, Wk, bk, Wv, bv):
    global LAST
    np8 = mybir.dt.np(FP8)
    x = np.asarray(x, np.float32)
    n = x.shape[0]
    r = n // M

    q = x @ np.asarray(Wq, np.float32).T + np.asarray(bq, np.float32)
    k = x @ np.asarray(Wk, np.float32).T + np.asarray(bk, np.float32)
    v = x @ np.asarray(Wv, np.float32).T                      # bias added at the end
    kn = k / np.maximum(np.linalg.norm(k, axis=1, keepdims=True), 1e-12)

    q8 = q.astype(np8)
    k8 = kn.astype(np8)
    v8q = v.astype(np8)

    kT8 = _to_dr(np.ascontiguousarray(k8.T))
    v8t = np.ascontiguousarray(v8q.reshape(n // 256, 2, 128, D).transpose(2, 0, 1, 3))

    in_maps = []
    for c in range(M):
        rows = slice(c * r, (c + 1) * r)
        in_maps.append({
            "qT8": _to_dr(np.ascontiguousarray(q8[rows].T)),
            "kT8": kT8,
            "v8": v8t,
        })
    res = run_bass_kernel_spmd(_get_nc(r), in_maps, core_ids=list(range(M)), trace=TRACE)
    LAST = res

    av = np.concatenate(
        [res.results[c]["av"].transpose(0, 2, 1, 3).reshape(r, D + 1) for c in range(M)],
        axis=0).astype(np.float32)
    num = av[:, 0:D]
    den = av[:, D]

    # Remove the diagonal term, emulating the device's fp8 rounding of the
    # relu'd score and of v so the subtraction cancels what was accumulated.
    sdiag = np.maximum((q8.astype(np.float32) * k8.astype(np.float32)).sum(axis=1), 0.0)
    wdiag = sdiag.astype(np8).astype(np.float32)
    num = num - wdiag[:, None] * v8q.astype(np.float32)
    den = den - wdiag

    out = num / np.maximum(den, 1e-12)[:, None] + x + np.asarray(bv, np.float32)
    return np.ascontiguousarray(out, dtype=np.float32)


# revision 4
# speedup vs baseline: 1.4087x; 1.0358x over previous
"""Row-sharded attention slab kernel, host-prepped fp8 Q/K/V.

Each of the 8 cores owns a [N/8, N] slab of the attention matrix.  The
host precomputes the three D x D projections in fp32 (O(N*D^2), ~5% of
total FLOPs), pre-normalizes k rows, and ships fp8 tensors: qT8 (own
rows), kT8 (all columns, pre-normalized), and v8 (row-major V).

The device does the O(N^2*D) work: score matmuls (fp8 DoubleRow),
ReLU + fp8 cast of the [N/8, N] score slab (split across ACT and DVE,
the true bottleneck), and the w@v accumulation into PSUM.  The row-sum
denominator accumulates in a separate 1-bank PSUM tile via tiny
wt @ ones matmuls, which frees enough PSUM banks to triple-buffer the
score tiles - without that, the WAR edge (score matmuls for group g+2
overwriting the tile relu(g) reads) serializes the pipeline.  Emission
is software-pipelined so those prefetch matmuls precede the w@v
matmuls of the current group in PE order.

The host then removes the diagonal term (emulating the device's fp8
rounding so the subtraction matches what the device accumulated),
divides by the row sums, and adds the residual x and the V bias (the
bias commutes out of the attention average because rows of w sum to 1).
"""

import numpy as np

import concourse.bass as bass
import concourse.bacc as bacc
import concourse.mybir as mybir
from concourse import tile
from concourse.bass_utils import run_bass_kernel_spmd

F32 = mybir.dt.float32
FP8 = mybir.dt.float8e4
AF = mybir.ActivationFunctionType
DR = mybir.MatmulPerfMode.DoubleRow

M = 8
N = 8192
D = 256

TRACE = False
LAST = None
_CACHE = {}

# ~47% of the 64 relu tiles go to DVE (1192ns/op) and the rest to ACT
# (1067ns/op) so both engines finish together.
N_DVE_RELU = 30


def _dve_set(total, n_dve):
    return {i for i in range(total) if (i * n_dve) // total != ((i + 1) * n_dve) // total}


def build(r=N // M):
    NP_ = N // 256           # 32 column pairs (v8 major dim)
    NCH = N // 1024          # 8 streaming chunks for kT8/v8
    NG = N // 512            # 16 score groups (4 col-blocks each) per row block
    RW = 256
    NRB = r // RW            # 4 row blocks

    nc = bacc.Bacc(None)
    qT8_d = nc.declare_dram_parameter("qT8", [128, 2, r], FP8, isOutput=False)
    kT8_d = nc.declare_dram_parameter("kT8", [128, 2, N], FP8, isOutput=False)
    v8_d = nc.declare_dram_parameter("v8", [128, NP_, 2, D], FP8, isOutput=False)
    av_d = nc.declare_dram_parameter("av", [NRB, 128, 2, D + 1], F32, isOutput=True)

    seq = [(rb, g) for rb in range(NRB) for g in range(NG)]
    dve_relu = _dve_set(len(seq), N_DVE_RELU)

    with tile.TileContext(nc, pool_alloc_mode="queue") as tc:
        with tc.tile_pool(name="pers", bufs=1) as pers, \
             tc.tile_pool(name="wtp", bufs=8) as wtp, \
             tc.tile_pool(name="avsp", bufs=2) as avsp, \
             tc.tile_pool(name="scp", bufs=3, space="PSUM") as scp, \
             tc.tile_pool(name="avp", bufs=1, space="PSUM") as avp, \
             tc.tile_pool(name="denp", bufs=1, space="PSUM") as denp:
            qT8 = pers.tile([128, 2, r], FP8, name="qT8", tag="qT8")
            kT8 = pers.tile([128, 2, N], FP8, name="kT8", tag="kT8")
            v8 = pers.tile([128, NP_, 2, D], FP8, name="v8", tag="v8")
            ones8 = pers.tile([128, 2, 1], FP8, name="ones8", tag="ones8")
            den = denp.tile([128, 2 * NRB], F32, name="den", tag="den")

            nc.vector.memset(ones8[:], 1.0)

            # Stream inputs on two issue queues (SP: kT8/qT8, Pool: v8) so
            # chunk delivery outpaces the ~570ns/group compute consumption;
            # small first chunks let group 0 start early.
            nc.sync.dma_start(kT8[:, :, 0:512], kT8_d[:, :, 0:512])
            nc.gpsimd.dma_start(v8[:, 0:2, :, :], v8_d[:, 0:2, :, :])
            nc.sync.dma_start(qT8[:, :, 0:RW], qT8_d[:, :, 0:RW])
            nc.gpsimd.dma_start(v8[:, 2:4, :, :], v8_d[:, 2:4, :, :])
            nc.sync.dma_start(kT8[:, :, 512:1024], kT8_d[:, :, 512:1024])
            nc.sync.dma_start(qT8[:, :, RW:r], qT8_d[:, :, RW:r])
            for ch in range(1, NCH):
                nc.sync.dma_start(kT8[:, :, ch * 1024:(ch + 1) * 1024],
                                  kT8_d[:, :, ch * 1024:(ch + 1) * 1024])
                nc.gpsimd.dma_start(v8[:, ch * 4:(ch + 1) * 4, :, :],
                                    v8_d[:, ch * 4:(ch + 1) * 4, :, :])

            sc_tiles = {}

            def emit_smm(i):
                rb, g = seq[i]
                sc = scp.tile([128, 1024], F32, name="sc", tag="sc")
                sc_tiles[i] = sc
                rsl = slice(rb * RW, (rb + 1) * RW)
                for t in range(4):
                    jb = g * 4 + t
                    nc.tensor.matmul(sc[:, t * 256:(t + 1) * 256],
                                     kT8[:, :, jb * 128:(jb + 1) * 128],
                                     qT8[:, :, rsl],
                                     start=True, stop=True, perf_mode=DR)

            avs = {}
            emit_smm(0)
            emit_smm(1)
            for i, (rb, g) in enumerate(seq):
                sc = sc_tiles.pop(i)
                wt = wtp.tile([128, 4, 256], FP8, name="wt", tag="wt")
                if i in dve_relu:
                    nc.vector.tensor_scalar_max(wt[:], sc[:], 0.0)
                else:
                    nc.scalar.activation(wt[:], sc[:], AF.Relu)
                if i + 2 < len(seq):
                    emit_smm(i + 2)
                if g == 0:
                    avs[rb] = avp.tile([128, 2, D], F32, name=f"av{rb}", tag="av")
                av = avs[rb]
                for pair in range(2):
                    jp = g * 2 + pair
                    st = (g == 0 and pair == 0)
                    sp = (g == NG - 1 and pair == 1)
                    for s in range(2):
                        wsl = wt[:, pair * 2:pair * 2 + 2, s * 128:(s + 1) * 128]
                        nc.tensor.matmul(av[:, s, :], wsl, v8[:, jp, :, :],
                                         start=st, stop=sp, perf_mode=DR)
                        nc.tensor.matmul(den[:, rb * 2 + s:rb * 2 + s + 1],
                                         wsl, ones8[:],
                                         start=st, stop=sp, perf_mode=DR)
                if g == NG - 1:
                    o = avsp.tile([128, 2, D + 1], F32, name=f"avs{rb}", tag="avs")
                    if rb % 2 == 0:
                        nc.vector.tensor_copy(o[:, :, 0:D], av[:])
                        nc.vector.tensor_copy(o[:, :, D], den[:, rb * 2:rb * 2 + 2])
                    else:
                        nc.scalar.activation(o[:, :, 0:D], av[:], AF.Copy)
                        nc.scalar.activation(o[:, :, D], den[:, rb * 2:rb * 2 + 2], AF.Copy)
                    nc.sync.dma_start(av_d[rb], o[:])
    nc.compile()
    return nc


def _get_nc(r=N // M):
    if r not in _CACHE:
        _CACHE[r] = build(r)
    return _CACHE[r]


def _to_dr(a2d):
    c, f = a2d.shape
    return np.ascontiguousarray(a2d.reshape(2, 128, f).transpose(1, 0, 2))


def kernel(x, Wq, bq, Wk, bk, Wv, bv):
    global LAST
    np8 = mybir.dt.np(FP8)
    x = np.asarray(x, np.float32)
    n = x.shape[0]
    r = n // M

    q = x @ np.asarray(Wq, np.float32).T + np.asarray(bq, np.float32)
    k = x @ np.asarray(Wk, np.float32).T + np.asarray(bk, np.float32)
    v = x @ np.asarray(Wv, np.float32).T                      # bias added at the end
    kn = k / np.maximum(np.linalg.norm(k, axis=1, keepdims=True), 1e-12)

    q8 = q.astype(np8)
    k8 = kn.astype(np8)
    v8q = v.astype(np8)

    kT8 = _to_dr(np.ascontiguousarray(k8.T))
    v8t = np.ascontiguousarray(v8q.reshape(n // 256, 2, 128, D).transpose(2, 0, 1, 3))

    in_maps = []
    for c in range(M):
        rows = slice(c * r, (c + 1) * r)
        in_maps.append({
            "qT8": _to_dr(np.ascontiguousarray(q8[rows].T)),
            "kT8": kT8,
            "v8": v8t,
        })
    res = run_bass_kernel_spmd(_get_nc(r), in_maps, core_ids=list(range(M)), trace=TRACE)
    LAST = res

    av = np.concatenate(
        [res.results[c]["av"].transpose(0, 2, 1, 3).reshape(r, D + 1) for c in range(M)],
        axis=0).astype(np.float32)
    num = av[:, 0:D]
    den = av[:, D]

    # Remove the diagonal term, emulating the device's fp8 rounding of the
    # relu'd score and of v so the subtraction cancels what was accumulated.
    sdiag = np.maximum((q8.astype(np.float32) * k8.astype(np.float32)).sum(axis=1), 0.0)
    wdiag = sdiag.astype(np8).astype(np.float32)
    num = num - wdiag[:, None] * v8q.astype(np.float32)
    den = den - wdiag

    out = num / np.maximum(den, 1e-12)[:, None] + x + np.asarray(bv, np.float32)
    return np.ascontiguousarray(out, dtype=np.float32)


# revision 7
# speedup vs baseline: 1.4936x; 1.0603x over previous
"""Row-sharded attention slab kernel, host-prepped fp8 Q/K/V.

Each of the 8 cores owns a [N/8, N] slab of the attention matrix.  The
host precomputes the three D x D projections in fp32 (O(N*D^2), ~5% of
total FLOPs), pre-normalizes k rows, and ships fp8 tensors: qT8 (own
rows), kT8 (all columns, pre-normalized), and v8 (row-major V).

The device does the O(N^2*D) work: score matmuls (fp8 DoubleRow),
ReLU + fp8 cast of the [N/8, N] score slab (split across ACT and DVE,
the true bottleneck), and the w@v accumulation into PSUM.  The row-sum
denominator accumulates in a separate 1-bank PSUM tile via tiny
wt @ ones matmuls, which frees enough PSUM banks to triple-buffer the
score tiles - without that, the WAR edge (score matmuls for group g+2
overwriting the tile relu(g) reads) serializes the pipeline.  Emission
is software-pipelined so those prefetch matmuls precede the w@v
matmuls of the current group in PE order.

The host then removes the diagonal term (emulating the device's fp8
rounding so the subtraction matches what the device accumulated),
divides by the row sums, and adds the residual x and the V bias (the
bias commutes out of the attention average because rows of w sum to 1).
"""

import numpy as np

import concourse.bass as bass
import concourse.bacc as bacc
import concourse.mybir as mybir
from concourse import tile
from concourse.bass_utils import run_bass_kernel_spmd

F32 = mybir.dt.float32
FP8 = mybir.dt.float8e4
AF = mybir.ActivationFunctionType
DR = mybir.MatmulPerfMode.DoubleRow

M = 8
N = 8192
D = 256

TRACE = False
LAST = None
_CACHE = {}

# ~47% of the 64 relu tiles go to DVE (1192ns/op) and the rest to ACT
# (1067ns/op) so both engines finish together.
N_DVE_RELU = 30


def _dve_set(total, n_dve):
    return {int(j * total / n_dve) for j in range(n_dve)}


def build(r=N // M):
    NP_ = N // 256           # 32 column pairs (v8 major dim)
    NCH = N // 1024          # 8 streaming chunks for kT8/v8
    NG = N // 512            # 16 score groups (4 col-blocks each) per row block
    RW = 256
    NRB = r // RW            # 4 row blocks

    nc = bacc.Bacc(None)
    qT8_d = nc.declare_dram_parameter("qT8", [128, 2, r], FP8, isOutput=False)
    kT8_d = nc.declare_dram_parameter("kT8", [128, 2, N], FP8, isOutput=False)
    v8_d = nc.declare_dram_parameter("v8", [128, NP_, 2, D], FP8, isOutput=False)
    av_d = nc.declare_dram_parameter("av", [NRB, 128, 2, D + 1], F32, isOutput=True)

    seq = [(rb, g) for rb in range(NRB) for g in range(NG)]
    dve_relu = _dve_set(len(seq), N_DVE_RELU)

    with tile.TileContext(nc, pool_alloc_mode="queue") as tc:
        with tc.tile_pool(name="pers", bufs=1) as pers, \
             tc.tile_pool(name="wtp", bufs=8) as wtp, \
             tc.tile_pool(name="avsp", bufs=2) as avsp, \
             tc.tile_pool(name="scp", bufs=3, space="PSUM") as scp, \
             tc.tile_pool(name="avp", bufs=1, space="PSUM") as avp, \
             tc.tile_pool(name="denp", bufs=1, space="PSUM") as denp:
            qT8 = pers.tile([128, 2, r], FP8, name="qT8", tag="qT8")
            kT8 = pers.tile([128, 2, N], FP8, name="kT8", tag="kT8")
            v8 = pers.tile([128, NP_, 2, D], FP8, name="v8", tag="v8")
            ones8 = pers.tile([128, 2, 1], FP8, name="ones8", tag="ones8")
            den = denp.tile([128, 2 * NRB], F32, name="den", tag="den")

            nc.vector.memset(ones8[:], 1.0)

            # Stream inputs on two issue queues (SP: kT8/qT8, Pool: v8) so
            # chunk delivery outpaces the ~570ns/group compute consumption;
            # small first chunks let group 0 start early.
            nc.sync.dma_start(kT8[:, :, 0:512], kT8_d[:, :, 0:512])
            nc.gpsimd.dma_start(v8[:, 0:2, :, :], v8_d[:, 0:2, :, :])
            nc.sync.dma_start(qT8[:, :, 0:RW], qT8_d[:, :, 0:RW])
            nc.gpsimd.dma_start(v8[:, 2:4, :, :], v8_d[:, 2:4, :, :])
            nc.sync.dma_start(kT8[:, :, 512:1024], kT8_d[:, :, 512:1024])
            nc.sync.dma_start(qT8[:, :, RW:r], qT8_d[:, :, RW:r])
            for ch in range(1, NCH):
                nc.sync.dma_start(kT8[:, :, ch * 1024:(ch + 1) * 1024],
                                  kT8_d[:, :, ch * 1024:(ch + 1) * 1024])
                nc.gpsimd.dma_start(v8[:, ch * 4:(ch + 1) * 4, :, :],
                                    v8_d[:, ch * 4:(ch + 1) * 4, :, :])

            sc_tiles = {}

            def emit_smm(i):
                rb, g = seq[i]
                sc = scp.tile([128, 1024], F32, name="sc", tag="sc")
                sc_tiles[i] = sc
                rsl = slice(rb * RW, (rb + 1) * RW)
                for t in range(4):
                    jb = g * 4 + t
                    nc.tensor.matmul(sc[:, t * 256:(t + 1) * 256],
                                     kT8[:, :, jb * 128:(jb + 1) * 128],
                                     qT8[:, :, rsl],
                                     start=True, stop=True, perf_mode=DR)

            avs = {}
            wts = {}

            def emit_wv(i):
                rb, g = seq[i]
                wt = wts.pop(i)
                if g == 0:
                    avs[rb] = avp.tile([128, 2, D], F32, name=f"av{rb}", tag="av")
                av = avs[rb]
                for pair in range(2):
                    jp = g * 2 + pair
                    st = (g == 0 and pair == 0)
                    sp = (g == NG - 1 and pair == 1)
                    for s in range(2):
                        wsl = wt[:, pair * 2:pair * 2 + 2, s * 128:(s + 1) * 128]
                        nc.tensor.matmul(av[:, s, :], wsl, v8[:, jp, :, :],
                                         start=st, stop=sp, perf_mode=DR)
                        nc.tensor.matmul(den[:, rb * 2 + s:rb * 2 + s + 1],
                                         wsl, ones8[:],
                                         start=st, stop=sp, perf_mode=DR)
                if g == NG - 1:
                    # split the evacuation across both engines to halve the
                    # latency before the next row block may reuse the av bank
                    o = avsp.tile([128, 2, D + 1], F32, name=f"avs{rb}", tag="avs")
                    nc.vector.tensor_copy(o[:, 0, 0:D], av[:, 0, :])
                    nc.vector.tensor_copy(o[:, 0, D:D + 1], den[:, rb * 2:rb * 2 + 1])
                    nc.scalar.activation(o[:, 1, 0:D], av[:, 1, :], AF.Copy)
                    nc.scalar.activation(o[:, 1, D:D + 1],
                                         den[:, rb * 2 + 1:rb * 2 + 2], AF.Copy)
                    nc.sync.dma_start(av_d[rb], o[:])

            WVLAG = 2  # keep PE's in-order queue free of stalled w@v matmuls
            emit_smm(0)
            emit_smm(1)
            for i, (rb, g) in enumerate(seq):
                sc = sc_tiles.pop(i)
                wt = wtp.tile([128, 4, 256], FP8, name="wt", tag="wt")
                wts[i] = wt
                if i in dve_relu:
                    nc.vector.tensor_scalar_max(wt[:], sc[:], 0.0)
                else:
                    nc.scalar.activation(wt[:], sc[:], AF.Relu)
                if i + 2 < len(seq):
                    emit_smm(i + 2)
                if i >= WVLAG:
                    emit_wv(i - WVLAG)
            for i in range(len(seq) - WVLAG, len(seq)):
                emit_wv(i)
    nc.compile()
    return nc


def _get_nc(r=N // M):
    if r not in _CACHE:
        _CACHE[r] = build(r)
    return _CACHE[r]


def _to_dr(a2d):
    c, f = a2d.shape
    return np.ascontiguousarray(a2d.reshape(2, 128, f).transpose(1, 0, 2))


def kernel(x, Wq, bq, Wk, bk, Wv, bv):
    global LAST
    np8 = mybir.dt.np(FP8)
    x = np.asarray(x, np.float32)
    n = x.shape[0]
    r = n // M

    q = x @ np.asarray(Wq, np.float32).T + np.asarray(bq, np.float32)
    k = x @ np.asarray(Wk, np.float32).T + np.asarray(bk, np.float32)
    v = x @ np.asarray(Wv, np.float32).T                      # bias added at the end
    kn = k / np.maximum(np.linalg.norm(k, axis=1, keepdims=True), 1e-12)

    q8 = q.astype(np8)
    k8 = kn.astype(np8)
    v8q = v.astype(np8)

    kT8 = _to_dr(np.ascontiguousarray(k8.T))
    v8t = np.ascontiguousarray(v8q.reshape(n // 256, 2, 128, D).transpose(2, 0, 1, 3))

    in_maps = []
    for c in range(M):
        rows = slice(c * r, (c + 1) * r)
        in_maps.append({
            "qT8": _to_dr(np.ascontiguousarray(q8[rows].T)),
            "kT8": kT8,
            "v8": v8t,
        })
    res = run_bass_kernel_spmd(_get_nc(r), in_maps, core_ids=list(range(M)), trace=TRACE)
    LAST = res

    av = np.concatenate(
        [res.results[c]["av"].transpose(0, 2, 1, 3).reshape(r, D + 1) for c in range(M)],
        axis=0).astype(np.float32)
    num = av[:, 0:D]
    den = av[:, D]

    # Remove the diagonal term, emulating the device's fp8 rounding of the
    # relu'd score and of v so the subtraction cancels what was accumulated.
    sdiag = np.maximum((q8.astype(np.float32) * k8.astype(np.float32)).sum(axis=1), 0.0)
    wdiag = sdiag.astype(np8).astype(np.float32)
    num = num - wdiag[:, None] * v8q.astype(np.float32)
    den = den - wdiag

    out = num / np.maximum(den, 1e-12)[:, None] + x + np.asarray(bv, np.float32)
    return np.ascontiguousarray(out, dtype=np.float32)
